# revision 8
# baseline (speedup 1.0000x reference)
"""Trainium2 Bass kernel for a pre-norm transformer block (MHA + MLP).

Sharding: sequence-parallel over 8 cores. Each core owns 512 tokens
(batch b = core//4, token block core%4). All weights are replicated.
The only collectives are two 4-rank AllGathers (K^T and V) inside each
batch group, replacing Megatron-style AllReduces (4 MB vs 16 MB payload).

Dataflow is feature-major (channels on partitions, tokens on the free
axis) end-to-end, so no on-chip transposes are needed:
  - LN mean/var via ones-matmul partition reduction on the TensorEngine
  - scores S^T[k, q] per head with softmax over the partition (k) axis:
    exp is fused into the PSUM->SBUF move on the ScalarEngine, and the
    softmax denominator comes free from an appended ones-column in V
  - odd heads run as base-64 quadrant matmuls (PE tile_position)
  - matmuls run in float32r (11-bit mantissa, 1 cycle/row) with weights
    pre-rounded on the host; the residual path stays exact fp32
"""
import sys

sys.path.insert(0, "/opt/trn_rl_repo")
import numpy as np
import concourse.bass as bass
import concourse.mybir as mybir
import concourse.tile as tile
from concourse import bacc
from concourse.bass_utils import run_bass_kernel_spmd

# problem shapes (hardcoded per contract)
B, N, D = 2, 2048, 1024
H, DH = 16, 64
HID = 4096
NCORES = 8
TOK = (B * N) // NCORES  # 512 tokens per core
EPS = 1e-5
SCALE = DH**-0.5
P = 128
CH = D // P  # 8 channel chunks of the model dim
KC = N // P  # 16 key chunks of the full sequence
HCH = HID // P  # 32 hidden chunks
RANKS = 4  # per-batch replica group size

F32 = mybir.dt.float32
F32R = mybir.dt.float32r
AF = mybir.ActivationFunctionType
OP = mybir.AluOpType

REPLICA_GROUPS = [[0, 1, 2, 3], [4, 5, 6, 7]]


def round_fp32r(x: np.ndarray) -> np.ndarray:
    """Round fp32 to fp32r (8-bit exp, 11-bit mantissa, RNE) on host."""
    u = np.ascontiguousarray(x, dtype=np.float32).view(np.uint32)
    u = (u + 0x7FF + ((u >> 12) & 1)) & np.uint32(0xFFFFF000)
    return u.view(np.float32)


def _ln_stripe(v: np.ndarray) -> np.ndarray:
    """[D] per-channel vector -> [P, D//P] feature-major stripe (c = ch*128+p)."""
    return np.ascontiguousarray(np.asarray(v).reshape(-1, P).T.astype(np.float32))


def build_program():
    nc = bacc.Bacc("TRN2", target_bir_lowering=False, debug=False, num_devices=NCORES)

    # ---- kernel I/O ----
    xT = nc.dram_tensor("xT", [D, TOK], F32, kind="ExternalInput").ap()
    qkv_wT = nc.dram_tensor("qkv_wT", [D, 3 * D], F32R, kind="ExternalInput").ap()
    proj_wT = nc.dram_tensor("proj_wT", [D, D], F32R, kind="ExternalInput").ap()
    fc1_wT = nc.dram_tensor("fc1_wT", [D, HID], F32R, kind="ExternalInput").ap()
    fc2_wT = nc.dram_tensor("fc2_wT", [HID, D], F32R, kind="ExternalInput").ap()
    ln1g = nc.dram_tensor("ln1g", [P, CH], F32, kind="ExternalInput").ap()
    ln1b = nc.dram_tensor("ln1b", [P, CH], F32, kind="ExternalInput").ap()
    ln2g = nc.dram_tensor("ln2g", [P, CH], F32, kind="ExternalInput").ap()
    ln2b = nc.dram_tensor("ln2b", [P, CH], F32, kind="ExternalInput").ap()
    projb = nc.dram_tensor("projb", [P, CH], F32, kind="ExternalInput").ap()
    fc1b = nc.dram_tensor("fc1b", [P, HCH], F32, kind="ExternalInput").ap()
    fc2b = nc.dram_tensor("fc2b", [P, CH], F32, kind="ExternalInput").ap()
    outT = nc.dram_tensor("outT", [D, TOK], F32, kind="ExternalOutput").ap()

    xT_chunks = xT.rearrange("(ch p) t -> p ch t", p=P)

    with tile.TileContext(nc) as tc:
        with (
            tc.tile_pool(name="consts", bufs=1) as consts,
            tc.tile_pool(name="bigs", bufs=1) as bigs,
            tc.tile_pool(name="work", bufs=3) as work,
            tc.tile_pool(name="wpool", bufs=3) as wpool,
            tc.tile_pool(name="kv", bufs=2) as kvpool,
            tc.tile_pool(name="pp", bufs=4) as ppool,
            tc.tile_pool(name="rows", bufs=4) as rows,
            tc.tile_pool(name="bc", bufs=2) as bcpool,
            tc.tile_pool(name="stg", bufs=3) as stg,
            tc.tile_pool(name="ps_acc", bufs=4, space="PSUM") as ps_acc,
            tc.tile_pool(name="ps_ctx", bufs=2, space="PSUM") as ps_ctx,
            tc.tile_pool(name="ps_row", bufs=2, space="PSUM") as ps_row,
            tc.tile_pool(name="dram", bufs=1, space="DRAM") as dram,
        ):
            # ---- constants ----
            ones_r = consts.tile([P, 1], F32R)
            nc.vector.memset(ones_r[:].bitcast(F32), 1.0)
            eps_row = consts.tile([1, 1], F32, tag="eps")
            nc.vector.memset(eps_row[:], EPS)
            ln1g_sb = consts.tile([P, CH], F32, tag="ln1g")
            ln1b_sb = consts.tile([P, CH], F32, tag="ln1b")
            ln2g_sb = consts.tile([P, CH], F32, tag="ln2g")
            ln2b_sb = consts.tile([P, CH], F32, tag="ln2b")
            projb_sb = consts.tile([P, CH], F32, tag="projb")
            fc1b_sb = consts.tile([P, HCH], F32, tag="fc1b")
            fc2b_sb = consts.tile([P, CH], F32, tag="fc2b")
            for t_, s_ in (
                (ln1g_sb, ln1g),
                (ln1b_sb, ln1b),
                (ln2g_sb, ln2g),
                (ln2b_sb, ln2b),
                (projb_sb, projb),
                (fc1b_sb, fc1b),
                (fc2b_sb, fc2b),
            ):
                nc.sync.dma_start(t_[:], s_[:])

            # ---- collective DRAM buffers ----
            k_cc_in = dram.tile([D, TOK], F32R, tag="kin")
            k_cc_out = dram.tile([RANKS * D, TOK], F32R, tag="kout")
            v_cc_in = dram.tile([TOK, D], F32R, tag="vin")
            v_cc_out = dram.tile([RANKS * TOK, D], F32R, tag="vout")

            def layer_norm(get_chunk, g_sb, b_sb, dst):
                """dst[:,ch,:] = (x - mu)*rsqrt(var+eps)*g + b over channels.
                get_chunk(ch) -> [P, TOK] fp32 SBUF AP (may DMA-load)."""
                psum_mu = ps_row.tile([1, TOK], F32, tag="row")
                psum_s2 = ps_row.tile([1, TOK], F32, tag="row")
                for ch in range(CH):
                    src = get_chunk(ch)
                    nc.tensor.matmul(
                        psum_mu[:],
                        ones_r[:].bitcast(F32),
                        src,
                        start=(ch == 0),
                        stop=(ch == CH - 1),
                    )
                    sq = work.tile([P, TOK], F32R, tag="sq")
                    nc.vector.tensor_mul(out=sq[:], in0=src, in1=src)
                    nc.tensor.matmul(
                        psum_s2[:],
                        ones_r[:],
                        sq[:],
                        start=(ch == 0),
                        stop=(ch == CH - 1),
                    )
                mu = rows.tile([1, TOK], F32, tag="r")
                nc.vector.tensor_scalar_mul(mu[:], psum_mu[:], 1.0 / D)
                var = rows.tile([1, TOK], F32, tag="r")
                nc.vector.tensor_tensor(var[:], mu[:], mu[:], OP.mult)
                ex2 = rows.tile([1, TOK], F32, tag="r")
                nc.vector.tensor_scalar_mul(ex2[:], psum_s2[:], 1.0 / D)
                nc.vector.tensor_sub(var[:], ex2[:], var[:])
                rstd = rows.tile([1, TOK], F32, tag="r")
                nc.scalar.activation(
                    out=rstd[:], in_=var[:], func=AF.Sqrt, bias=eps_row[:]
                )
                nc.vector.reciprocal(rstd[:], rstd[:])
                cneg = rows.tile([1, TOK], F32, tag="r")
                nc.vector.tensor_tensor(cneg[:], mu[:], rstd[:], OP.mult)
                nc.vector.tensor_scalar_mul(cneg[:], cneg[:], -1.0)
                rstd_b = bcpool.tile([P, TOK], F32, tag="bc")
                nc.gpsimd.partition_broadcast(rstd_b[:], rstd[:])
                c_b = bcpool.tile([P, TOK], F32, tag="bc")
                nc.gpsimd.partition_broadcast(c_b[:], cneg[:])
                for ch in range(CH):
                    src = get_chunk(ch)
                    t1 = work.tile([P, TOK], F32, tag="t1")
                    nc.vector.tensor_mul(t1[:], src, rstd_b[:])
                    nc.vector.tensor_add(t1[:], t1[:], c_b[:])
                    nc.vector.tensor_scalar(
                        out=dst[:, ch, :],
                        in0=t1[:],
                        scalar1=g_sb[:, ch : ch + 1],
                        scalar2=b_sb[:, ch : ch + 1],
                        op0=OP.mult,
                        op1=OP.add,
                    )

            # ---- stage 1: LN1 (x streamed from DRAM) ----
            h1 = bigs.tile([P, CH, TOK], F32R, tag="h12")

            def ln1_chunk(ch):
                xc = work.tile([P, TOK], F32, tag="xc")
                nc.sync.dma_start(xc[:], xT_chunks[:, ch, :])
                return xc[:]

            layer_norm(ln1_chunk, ln1g_sb, ln1b_sb, h1)

            # ---- stage 2: QKV (K first so its AllGather overlaps Q/V) ----
            def qkv_mtile(cols):
                w = wpool.tile([P, CH, P], F32R, tag="wcol8")
                nc.sync.dma_start(
                    w[:], qkv_wT[:, cols].rearrange("(ch p) o -> p ch o", p=P)
                )
                psum = ps_acc.tile([P, TOK], F32, tag="acc")
                for ch in range(CH):
                    nc.tensor.matmul(
                        psum[:],
                        w[:, ch, :],
                        h1[:, ch, :],
                        start=(ch == 0),
                        stop=(ch == CH - 1),
                    )
                return psum

            # K^T tiles -> DRAM collective input
            for m in range(CH):
                psum = qkv_mtile(slice(D + m * P, D + (m + 1) * P))
                ktmp = stg.tile([P, TOK], F32R, tag="cp")
                nc.vector.tensor_copy(out=ktmp[:], in_=psum[:])
                nc.sync.dma_start(k_cc_in[m * P : (m + 1) * P, :], ktmp[:])
            nc.gpsimd.collective_compute(
                "AllGather",
                OP.bypass,
                ins=[k_cc_in[:].opt()],
                outs=[k_cc_out[:].opt()],
                replica_groups=REPLICA_GROUPS,
            )

            # Q^T tiles stay in SBUF (feature-major, heads in half-partitions)
            qfullT = bigs.tile([P, CH, TOK], F32R, tag="qg8")
            for m in range(CH):
                psum = qkv_mtile(slice(m * P, (m + 1) * P))
                nc.vector.tensor_copy(out=qfullT[:, m, :], in_=psum[:])

            # V token-major [t, vout]: lhsT = h1 chunk, rhs = W_v columns
            for vh in range(2):
                wv = wpool.tile([P, CH, 512], F32R, tag="w8", bufs=2)
                nc.sync.dma_start(
                    wv[:],
                    qkv_wT[:, 2 * D + vh * 512 : 2 * D + (vh + 1) * 512].rearrange(
                        "(ch p) o -> p ch o", p=P
                    ),
                )
                for tt in range(TOK // P):
                    psum = ps_acc.tile([P, TOK], F32, tag="acc")
                    for ch in range(CH):
                        nc.tensor.matmul(
                            psum[:],
                            h1[:, ch, tt * P : (tt + 1) * P],
                            wv[:, ch, :],
                            start=(ch == 0),
                            stop=(ch == CH - 1),
                        )
                    vtmp = stg.tile([P, TOK], F32R, tag="cp")
                    nc.vector.tensor_copy(out=vtmp[:], in_=psum[:])
                    nc.sync.dma_start(
                        v_cc_in[tt * P : (tt + 1) * P, vh * 512 : (vh + 1) * 512],
                        vtmp[:],
                    )
            nc.gpsimd.collective_compute(
                "AllGather",
                OP.bypass,
                ins=[v_cc_in[:].opt()],
                outs=[v_cc_out[:].opt()],
                replica_groups=REPLICA_GROUPS,
            )

            # ---- stage 3: attention (head pairs share a kpair tile) ----
            v_view = v_cc_out[:].rearrange(
                "(r tc p) (hh d) -> r tc p hh d", r=RANKS, tc=TOK // P, hh=H
            )
            ctxT = bigs.tile([P, CH, TOK], F32R, tag="ctxacc")
            for hp in range(H // 2):
                # K^T for heads (2hp, 2hp+1): 128 consecutive rows per rank
                kp = kvpool.tile([P, RANKS, TOK // P, P], F32R, tag="kp")
                nc.sync.dma_start(
                    kp[:],
                    k_cc_out[:]
                    .rearrange("(r c p) t -> r c p t", r=RANKS, c=CH)[:, hp, :, :]
                    .rearrange("r p (tc tk) -> p r tc tk", tk=P),
                )
                for h in (2 * hp, 2 * hp + 1):
                    half = slice((h % 2) * 64, (h % 2) * 64 + 64)
                    vf = kvpool.tile([P, KC, DH + 1], F32R, tag="vf", bufs=3)
                    nc.sync.dma_start(
                        vf[:, :, 0:DH].rearrange("p (r tc) d -> p r tc d", r=RANKS),
                        v_view[:, :, :, h, :].rearrange("r tc p d -> p r tc d"),
                    )
                    nc.vector.memset(vf[:, :, DH : DH + 1].bitcast(F32), 1.0)
                    psum_c = ps_ctx.tile([DH + 1, TOK], F32, tag="ctx")
                    for kc in range(KC):
                        r, tcc = divmod(kc, TOK // P)
                        ps_s = ps_acc.tile([P, TOK], F32, tag="acc")
                        nc.tensor.matmul(
                            ps_s[:],
                            kp[half, r, tcc, :],
                            qfullT[half, h // 2, :],
                            start=True,
                            stop=True,
                        )
                        pt = ppool.tile([P, TOK], F32R, tag="p")
                        nc.scalar.activation(
                            out=pt[:], in_=ps_s[:], func=AF.Exp, scale=SCALE
                        )
                        nc.tensor.matmul(
                            psum_c[:],
                            vf[:, kc, :],
                            pt[:],
                            start=(kc == 0),
                            stop=(kc == KC - 1),
                        )
                    rrow = rows.tile([1, TOK], F32, tag="r")
                    nc.vector.reciprocal(rrow[:], psum_c[DH : DH + 1, :])
                    rb = bcpool.tile([64, TOK], F32, tag="rb", bufs=3)
                    nc.gpsimd.partition_broadcast(rb[:], rrow[:])
                    nc.vector.tensor_tensor(
                        ctxT[half, h // 2, :], psum_c[0:DH, :], rb[:], OP.mult
                    )

            # ---- stage 4: proj + residual ----
            x2 = bigs.tile([P, CH, TOK], F32, tag="x2")
            for m in range(CH):
                w = wpool.tile([P, CH, P], F32R, tag="wcol8")
                nc.sync.dma_start(
                    w[:],
                    proj_wT[:, m * P : (m + 1) * P].rearrange(
                        "(ch p) o -> p ch o", p=P
                    ),
                )
                psum = ps_acc.tile([P, TOK], F32, tag="acc")
                for ch in range(CH):
                    nc.tensor.matmul(
                        psum[:],
                        w[:, ch, :],
                        ctxT[:, ch, :],
                        start=(ch == 0),
                        stop=(ch == CH - 1),
                    )
                attn_sb = stg.tile([P, TOK], F32, tag="stg", bufs=2)
                nc.scalar.activation(
                    out=attn_sb[:],
                    in_=psum[:],
                    func=AF.Identity,
                    bias=projb_sb[:, m : m + 1],
                )
                xc = work.tile([P, TOK], F32, tag="xc")
                nc.sync.dma_start(xc[:], xT_chunks[:, m, :])
                nc.vector.tensor_add(out=x2[:, m, :], in0=attn_sb[:], in1=xc[:])

            # ---- stage 5: LN2 ----
            h2 = bigs.tile([P, CH, TOK], F32R, tag="h12")
            layer_norm(lambda ch: x2[:, ch, :], ln2g_sb, ln2b_sb, h2)

            # ---- stage 6: MLP in hidden-quarters with SBUF accumulator ----
            acc_sb = bigs.tile([P, CH, TOK], F32, tag="ctxacc")
            QH = 8  # hidden chunks per quarter
            for q in range(HCH // QH):
                g8 = bigs.tile([P, QH, TOK], F32R, tag="qg8")
                for mm in range(QH):
                    m = q * QH + mm
                    w = wpool.tile([P, CH, P], F32R, tag="wcol8")
                    nc.sync.dma_start(
                        w[:],
                        fc1_wT[:, m * P : (m + 1) * P].rearrange(
                            "(ch p) o -> p ch o", p=P
                        ),
                    )
                    psum = ps_acc.tile([P, TOK], F32, tag="acc")
                    for ch in range(CH):
                        nc.tensor.matmul(
                            psum[:],
                            w[:, ch, :],
                            h2[:, ch, :],
                            start=(ch == 0),
                            stop=(ch == CH - 1),
                        )
                    nc.scalar.activation(
                        out=g8[:, mm, :],
                        in_=psum[:],
                        func=AF.Gelu,
                        bias=fc1b_sb[:, m : m + 1],
                    )
                for m2 in range(CH):
                    w2 = wpool.tile([P, QH, P], F32R, tag="w8", bufs=2)
                    nc.sync.dma_start(
                        w2[:],
                        fc2_wT[
                            q * QH * P : (q + 1) * QH * P, m2 * P : (m2 + 1) * P
                        ].rearrange("(hc p) o -> p hc o", p=P),
                    )
                    psum = ps_acc.tile([P, TOK], F32, tag="acc")
                    for hc in range(QH):
                        nc.tensor.matmul(
                            psum[:],
                            w2[:, hc, :],
                            g8[:, hc, :],
                            start=(hc == 0),
                            stop=(hc == QH - 1),
                        )
                    if q == 0:
                        nc.vector.tensor_copy(out=acc_sb[:, m2, :], in_=psum[:])
                    else:
                        nc.vector.tensor_add(
                            out=acc_sb[:, m2, :], in0=acc_sb[:, m2, :], in1=psum[:]
                        )
            for m2 in range(CH):
                o_sb = stg.tile([P, TOK], F32, tag="stg", bufs=2)
                nc.scalar.activation(
                    out=o_sb[:],
                    in_=acc_sb[:, m2, :],
                    func=AF.Identity,
                    bias=fc2b_sb[:, m2 : m2 + 1],
                )
                o_f = stg.tile([P, TOK], F32, tag="of", bufs=2)
                nc.vector.tensor_add(out=o_f[:], in0=o_sb[:], in1=x2[:, m2, :])
                nc.sync.dma_start(outT[m2 * P : (m2 + 1) * P, :], o_f[:])

    nc.compile()
    return nc


_CACHE = {}


def _get_program():
    if "nc" not in _CACHE:
        _CACHE["nc"] = build_program()
    return _CACHE["nc"]


def _prep_inputs(inputs):
    """Host-side sharding + layout prep. Returns per-core in_maps."""
    x = np.asarray(inputs["x"], dtype=np.float32)
    shared = {
        "qkv_wT": round_fp32r(np.asarray(inputs["qkv_w"], np.float32).T),
        "proj_wT": round_fp32r(np.asarray(inputs["proj_w"], np.float32).T),
        "fc1_wT": round_fp32r(np.asarray(inputs["fc1_w"], np.float32).T),
        "fc2_wT": round_fp32r(np.asarray(inputs["fc2_w"], np.float32).T),
        "ln1g": _ln_stripe(inputs["ln1_g"]),
        "ln1b": _ln_stripe(inputs["ln1_b"]),
        "ln2g": _ln_stripe(inputs["ln2_g"]),
        "ln2b": _ln_stripe(inputs["ln2_b"]),
        "projb": _ln_stripe(inputs["proj_b"]),
        "fc1b": _ln_stripe(inputs["fc1_b"]),
        "fc2b": _ln_stripe(inputs["fc2_b"]),
    }
    in_maps = []
    for c in range(NCORES):
        b, blk = divmod(c, RANKS)
        xblk = x[b, blk * TOK : (blk + 1) * TOK, :]  # [TOK, D]
        m = dict(shared)
        m["xT"] = np.ascontiguousarray(xblk.T)  # [D, TOK]
        in_maps.append(m)
    return in_maps


def _assemble(results):
    out = np.empty((B, N, D), dtype=np.float32)
    for c in range(NCORES):
        b, blk = divmod(c, RANKS)
        out[b, blk * TOK : (blk + 1) * TOK, :] = results[c]["outT"].T
    return out


def run_device(inputs, **kwargs):
    nc = _get_program()
    in_maps = _prep_inputs(inputs)
    res = run_bass_kernel_spmd(nc, in_maps, core_ids=list(range(NCORES)), **kwargs)
    return _assemble(res.results), res


def kernel(**inputs) -> np.ndarray:
    out, _ = run_device(inputs)
    return out


# revision 34
# speedup vs baseline: 1.5682x; 1.5682x over previous
"""Trainium2 Bass kernel for a pre-norm transformer block (MHA + MLP).

Sharding: sequence-parallel over 8 cores. Each core owns 512 tokens
(batch b = core//4, token block core%4). All weights are replicated.
The only collectives are two 4-rank AllGathers (K^T and V) inside each
batch group, replacing Megatron-style AllReduces (4 MB vs 16 MB payload).

Dataflow is feature-major (channels on partitions, tokens on the free
axis) end-to-end, so no on-chip transposes are needed:
  - LN mean/var via ones-matmul partition reduction on the TensorEngine
  - scores S^T[k, q] per head with softmax over the partition (k) axis:
    exp is fused into the PSUM->SBUF move on the ScalarEngine, and the
    softmax denominator comes free from an appended ones-column in V
  - odd heads run as base-64 quadrant matmuls (PE tile_position)
  - matmuls run in float32r (11-bit mantissa, 1 cycle/row) with weights
    pre-rounded on the host; the residual path stays exact fp32
"""
import sys

sys.path.insert(0, "/opt/trn_rl_repo")
import numpy as np
import concourse.bass as bass
import concourse.mybir as mybir
import concourse.tile as tile
from concourse import bacc
from concourse.bass_utils import run_bass_kernel_spmd

# problem shapes (hardcoded per contract)
B, N, D = 2, 2048, 1024
H, DH = 16, 64
HID = 4096
NCORES = 8
TOK = (B * N) // NCORES  # 512 tokens per core
EPS = 1e-5
SCALE = DH**-0.5
P = 128
CH = D // P  # 8 channel chunks of the model dim
KC = N // P  # 16 key chunks of the full sequence
HCH = HID // P  # 32 hidden chunks
RANKS = 4  # per-batch replica group size

F32 = mybir.dt.float32
F32R = mybir.dt.float32r
AF = mybir.ActivationFunctionType
OP = mybir.AluOpType

REPLICA_GROUPS = [[0, 1, 2, 3], [4, 5, 6, 7]]


def round_fp32r(x: np.ndarray) -> np.ndarray:
    """Round fp32 to fp32r (8-bit exp, 11-bit mantissa, RNE) on host."""
    u = np.ascontiguousarray(x, dtype=np.float32).view(np.uint32)
    u = (u + 0x7FF + ((u >> 12) & 1)) & np.uint32(0xFFFFF000)
    return u.view(np.float32)


def _ln_stripe(v: np.ndarray) -> np.ndarray:
    """[D] per-channel vector -> [P, D//P] feature-major stripe (c = ch*128+p)."""
    return np.ascontiguousarray(np.asarray(v).reshape(-1, P).T.astype(np.float32))


ALL_STAGES = ("ln1", "qkv", "cc", "attn", "exp", "proj", "ln2", "mlp")


def build_program(stages=None, do_compile=True):
    if stages is None:
        stages = set(ALL_STAGES)
    stages = set(stages)
    nc = bacc.Bacc("TRN2", target_bir_lowering=False, debug=False, num_devices=NCORES)

    # ---- kernel I/O ----
    xT = nc.dram_tensor("xT", [D, TOK], F32, kind="ExternalInput").ap()
    qkv_wT = nc.dram_tensor("qkv_wT", [D, 3 * D], F32R, kind="ExternalInput").ap()
    proj_wT = nc.dram_tensor("proj_wT", [D, D], F32R, kind="ExternalInput").ap()
    fc1_wT = nc.dram_tensor("fc1_wT", [D, HID], F32R, kind="ExternalInput").ap()
    fc2_wT = nc.dram_tensor("fc2_wT", [HID, D], F32R, kind="ExternalInput").ap()
    ln1g = nc.dram_tensor("ln1g", [P, CH], F32, kind="ExternalInput").ap()
    ln1b = nc.dram_tensor("ln1b", [P, CH], F32, kind="ExternalInput").ap()
    ln2g = nc.dram_tensor("ln2g", [P, CH], F32, kind="ExternalInput").ap()
    ln2b = nc.dram_tensor("ln2b", [P, CH], F32, kind="ExternalInput").ap()
    projb = nc.dram_tensor("projb", [P, CH], F32, kind="ExternalInput").ap()
    fc1b = nc.dram_tensor("fc1b", [P, HCH], F32, kind="ExternalInput").ap()
    fc1mg = nc.dram_tensor("fc1mg", [P, HCH], F32, kind="ExternalInput").ap()
    fc2b = nc.dram_tensor("fc2b", [P, CH], F32, kind="ExternalInput").ap()
    outT = nc.dram_tensor("outT", [D, TOK], F32, kind="ExternalOutput").ap()

    xT_chunks = xT.rearrange("(ch p) t -> p ch t", p=P)

    with tile.TileContext(nc) as tc:
        with (
            tc.tile_pool(name="consts", bufs=1) as consts,
            tc.tile_pool(name="bigs", bufs=1) as bigs,
            tc.tile_pool(name="work", bufs=3) as work,
            tc.tile_pool(name="wpool", bufs=5) as wpool,
            tc.tile_pool(name="kv", bufs=2) as kvpool,
            tc.tile_pool(name="pp", bufs=2) as ppool,
            tc.tile_pool(name="rows", bufs=3) as rows,
            tc.tile_pool(name="bc", bufs=2) as bcpool,
            tc.tile_pool(name="stg", bufs=3) as stg,
            tc.tile_pool(name="dram", bufs=1, space="DRAM") as dram,
        ):
            # ---- constants ----
            ones_r = consts.tile([P, 1], F32R)
            nc.vector.memset(ones_r[:].bitcast(F32), 1.0)
            eps_row = consts.tile([1, 1], F32, tag="eps")
            nc.vector.memset(eps_row[:], EPS)
            ln1g_sb = consts.tile([P, CH], F32, tag="ln1g")
            ln1b_sb = consts.tile([P, CH], F32, tag="ln1b")
            ln2g_sb = consts.tile([P, CH], F32, tag="ln2g")
            ln2b_sb = consts.tile([P, CH], F32, tag="ln2b")
            projb_sb = consts.tile([P, CH], F32, tag="projb")
            fc1b_sb = consts.tile([P, HCH], F32, tag="fc1b")
            fc1mg_sb = consts.tile([P, HCH], F32, tag="fc1mg")
            fc2b_sb = consts.tile([P, CH], F32, tag="fc2b")

            # ---- collective DRAM buffers ----
            kv_in0 = dram.tile([D, TOK], F32R, tag="kvin0")
            kv_in1 = dram.tile([D, TOK], F32R, tag="kvin1")
            kv_out0 = dram.tile([RANKS * D, TOK], F32R, tag="kvout0")
            kv_out1 = dram.tile([RANKS * D, TOK], F32R, tag="kvout1")
            kv_ins = (kv_in0, kv_in1)
            kv_outs = (kv_out0, kv_out1)

            def layer_norm(get_chunk, g_sb, b_sb, dst, ps_row, xr_dst=None):
                psum_mu = ps_row.tile([1, TOK], F32, tag="row")
                psum_s2 = ps_row.tile([1, TOK], F32, tag="row")
                for ch in range(CH):
                    src = get_chunk(ch)
                    if xr_dst is None:
                        xrt = work.tile([P, TOK], F32R, tag="xr", name=f"xr_{ch}")
                        xr = xrt[:]
                    else:
                        xr = xr_dst[:, ch, :]
                    nc.gpsimd.tensor_copy(out=xr, in_=src)
                    nc.tensor.matmul(
                        psum_mu[:],
                        ones_r[:],
                        xr,
                        start=(ch == 0),
                        stop=(ch == CH - 1),
                    )
                    sq = work.tile([P, TOK], F32R, tag="sq")
                    nc.vector.tensor_mul(out=sq[:], in0=src, in1=src)
                    nc.tensor.matmul(
                        psum_s2[:],
                        ones_r[:],
                        sq[:],
                        start=(ch == 0),
                        stop=(ch == CH - 1),
                    )
                mu = rows.tile([1, TOK], F32, tag="r")
                nc.vector.tensor_scalar_mul(mu[:], psum_mu[:], 1.0 / D)
                var = rows.tile([1, TOK], F32, tag="r")
                nc.vector.tensor_tensor(var[:], mu[:], mu[:], OP.mult)
                ex2 = rows.tile([1, TOK], F32, tag="r")
                nc.vector.tensor_scalar_mul(ex2[:], psum_s2[:], 1.0 / D)
                nc.vector.tensor_sub(var[:], ex2[:], var[:])
                rstd = rows.tile([1, TOK], F32, tag="r")
                nc.scalar.activation(
                    out=rstd[:], in_=var[:], func=AF.Sqrt, bias=eps_row[:]
                )
                nc.vector.reciprocal(rstd[:], rstd[:])
                cpos = rows.tile([1, TOK], F32, tag="r")
                nc.vector.tensor_tensor(cpos[:], mu[:], rstd[:], OP.mult)
                rstd_b = bcpool.tile([P, TOK], F32, tag="bc")
                nc.gpsimd.partition_broadcast(rstd_b[:], rstd[:])
                c_b = bcpool.tile([P, TOK], F32, tag="bc")
                nc.gpsimd.partition_broadcast(c_b[:], cpos[:])
                if dst is None:
                    return rstd_b, c_b
                for ch in range(CH):
                    src = get_chunk(ch)
                    t1 = work.tile([P, TOK], F32, tag="t1")
                    nc.vector.tensor_mul(t1[:], src, rstd_b[:])
                    nc.vector.tensor_sub(t1[:], t1[:], c_b[:])
                    nc.scalar.activation(
                        out=dst[:, ch, :],
                        in_=t1[:],
                        func=AF.Identity,
                        bias=b_sb[:, ch : ch + 1],
                        scale=g_sb[:, ch : ch + 1],
                    )

            # ---- stage 1: LN1 (x streamed from DRAM) ----
            for t_, s_ in (
                (ln1g_sb, ln1g),
                (ln1b_sb, ln1b),
                (ln2g_sb, ln2g),
                (ln2b_sb, ln2b),
                (projb_sb, projb),
                (fc1b_sb, fc1b),
                (fc1mg_sb, fc1mg),
                (fc2b_sb, fc2b),
            ):
                nc.gpsimd.dma_start(t_[:], s_[:])
            h1 = bigs.tile([P, CH, TOK], F32R, tag="h12")

            def ln1_chunk(ch):
                xc = work.tile([P, TOK], F32, tag="xc")
                nc.sync.dma_start(xc[:], xT_chunks[:, ch, :])
                return xc[:]

            if "ln1" in stages:
                with tc.tile_pool(name="ps_row1", bufs=2, space="PSUM") as prow:
                    layer_norm(ln1_chunk, ln1g_sb, ln1b_sb, h1, prow)

            # ---- stage 2: QKV (K first so its AllGather overlaps Q/V) ----
            st2_pool_cm = tc.tile_pool(name="ps_mm2", bufs=4, space="PSUM")
            ps_acc = st2_pool_cm.__enter__()

            # K^T tiles -> DRAM collective input (ch-outer over m-groups)
            def qkv_mgroup(ms, col0, consume):
                ws, psums = [], []
                for m in ms:
                    w = wpool.tile([P, CH, P], F32R, tag="wcol8")
                    nc.sync.dma_start(
                        w[:],
                        qkv_wT[:, col0 + m * P : col0 + (m + 1) * P].rearrange(
                            "(ch p) o -> p ch o", p=P
                        ),
                    )
                    ws.append(w)
                    psums.append(ps_acc.tile([P, TOK], F32, tag="acc", name=f"ps_{m}"))
                for ch in range(CH):
                    for i, m in enumerate(ms):
                        nc.tensor.matmul(
                            psums[i][:],
                            ws[i][:, ch, :],
                            h1[:, ch, :],
                            start=(ch == 0),
                            stop=(ch == CH - 1),
                        )
                for i, m in enumerate(ms):
                    consume(m, psums[i])

            def k_consume(m, psum):
                ktmp = stg.tile([P, TOK], F32R, tag="cp")
                nc.vector.tensor_copy(out=ktmp[:], in_=psum[:])
                nc.sync.dma_start(
                    kv_ins[m // 4][(m % 4) * P : (m % 4 + 1) * P, :], ktmp[:]
                )

            if "qkv" in stages:
                for g0 in range(4):
                    qkv_mgroup(range(g0, g0 + 1), D, k_consume)

            # V token-major [t, vout]: lhsT = h1 chunk, rhs = W_v columns
            def v_phase(vh):
                wvh = []
                for chh in range(2):
                    wv4 = wpool.tile(
                        [P, CH // 2, 512], F32R, tag="wv", bufs=2,
                        name=f"wv_{vh}_{chh}",
                    )
                    nc.sync.dma_start(
                        wv4[:],
                        qkv_wT[
                            chh * 512 : (chh + 1) * 512,
                            2 * D + vh * 512 : 2 * D + (vh + 1) * 512,
                        ].rearrange("(ch p) o -> p ch o", p=P),
                    )
                    wvh.append(wv4)
                for tt in range(TOK // P):
                    psum = ps_acc.tile([P, TOK], F32, tag="acc", name=f"psv{vh}_{tt}")
                    for ch in range(CH):
                        nc.tensor.matmul(
                            psum[:],
                            h1[:, ch, tt * P : (tt + 1) * P],
                            wvh[ch // 4][:, ch % 4, :],
                            start=(ch == 0),
                            stop=(ch == CH - 1),
                        )
                    vtmp = stg.tile([P, TOK], F32R, tag="cp", name=f"vtmp{vh}_{tt}")
                    nc.vector.tensor_copy(out=vtmp[:], in_=psum[:])
                    nc.sync.dma_start(
                        kv_ins[vh][TOK + tt * P : TOK + (tt + 1) * P, :],
                        vtmp[:],
                    )
                if "cc" in stages:
                    nc.gpsimd.collective_compute(
                        "AllGather",
                        OP.bypass,
                        ins=[kv_ins[vh][:].opt()],
                        outs=[kv_outs[vh][:].opt()],
                        replica_groups=REPLICA_GROUPS,
                    )

            if "qkv" in stages:
                v_phase(0)
                for g0 in range(4, CH):
                    qkv_mgroup(range(g0, g0 + 1), D, k_consume)
            # Q^T tiles stay in SBUF (feature-major, heads in half-partitions)
            qfullT = bigs.tile([P, CH, TOK], F32R, tag="qg8")

            def q_consume(m, psum):
                nc.vector.tensor_copy(out=qfullT[:, m, :], in_=psum[:])

            if "qkv" in stages:
                for g0 in range(CH):
                    qkv_mgroup(range(g0, g0 + 1), 0, q_consume)
            v_views = tuple(
                kv_outs[i][:].rearrange(
                    "(r u tc p) (hh d) -> r u tc p hh d",
                    r=RANKS,
                    u=2,
                    tc=TOK // P,
                    hh=H // 2,
                )[:, 1]
                for i in range(2)
            )
            def load_kp(hp):
                kp = kvpool.tile(
                    [P, RANKS, TOK // P, P], F32R, tag="kp", name=f"kp{hp}"
                )
                nc.gpsimd.dma_start(
                    kp[:],
                    kv_outs[hp // 4][:]
                    .rearrange("(r q p) t -> r q p t", r=RANKS, q=CH)[
                        :, hp % 4, :, :
                    ]
                    .rearrange("r p (tc tk) -> p r tc tk", tk=P),
                )
                return kp

            def load_vf(h):
                vf = kvpool.tile(
                    [P, KC, DH + 1], F32R, tag="vf", bufs=3, name=f"vf{h}"
                )
                vf4 = vf[:, :, 0:DH].rearrange("p (r tc) d -> p r tc d", r=RANKS)
                for r_ in range(RANKS):
                    nc.gpsimd.dma_start(
                        vf4[:, r_, :, :],
                        v_views[h // 8][r_, :, :, h % 8, :].rearrange(
                            "tc p d -> p tc d"
                        ),
                    )
                nc.vector.memset(vf[:, :, DH : DH + 1].bitcast(F32), 1.0)
                return vf

            pre_kp = load_kp(0) if "attn" in stages else None
            pre_vf = load_vf(0) if "attn" in stages else None
            if "qkv" in stages:
                v_phase(1)
            st2_pool_cm.__exit__(None, None, None)


            # ---- stage 3: attention (head pairs share a kpair tile) ----
            ctxT = bigs.tile([P, CH, TOK], F32R, tag="ctxacc")

            attn_pools = (
                tc.tile_pool(name="ps_s", bufs=2, space="PSUM"),
                tc.tile_pool(name="ps_ctx", bufs=2, space="PSUM"),
            )
            ps_spool = attn_pools[0].__enter__()
            ps_ctx = attn_pools[1].__enter__()
            for hp in range(H // 2) if "attn" in stages else ():
                # K^T for heads (2hp, 2hp+1): 128 consecutive rows per rank
                kp = pre_kp if hp == 0 else load_kp(hp)
                for h in (2 * hp, 2 * hp + 1):
                    half = slice((h % 2) * 64, (h % 2) * 64 + 64)
                    vf = pre_vf if h == 0 else load_vf(h)
                    psum_c = ps_ctx.tile([DH + 1, TOK], F32, tag="ctx")
                    kc0 = 0
                    for nb in (3, 3, 3, 3, 2, 2):
                        ps_s = ps_spool.tile([P, 3 * TOK], F32, tag="s")
                        for j in range(nb):
                            kc = kc0 + j
                            r, tcc = divmod(kc, TOK // P)
                            nc.tensor.matmul(
                                ps_s[:, j * TOK : (j + 1) * TOK],
                                kp[half, r, tcc, :],
                                qfullT[half, h // 2, :],
                                start=True,
                                stop=True,
                            )
                        pt = ppool.tile([P, 3 * TOK], F32R, tag="p")
                        if "exp" in stages:
                            nc.scalar.activation(
                                out=pt[:, : nb * TOK],
                                in_=ps_s[:, : nb * TOK],
                                func=AF.Exp,
                                scale=SCALE,
                            )
                        for j in range(nb):
                            kc = kc0 + j
                            nc.tensor.matmul(
                                psum_c[:],
                                vf[:, kc, :],
                                pt[:, j * TOK : (j + 1) * TOK],
                                start=(kc == 0),
                                stop=(kc == KC - 1),
                            )
                        kc0 += nb
                    rrow = rows.tile([1, TOK], F32, tag="r")
                    nc.vector.reciprocal(rrow[:], psum_c[DH : DH + 1, :])
                    rb = bcpool.tile([64, TOK], F32, tag="rb", bufs=3)
                    nc.gpsimd.partition_broadcast(rb[:], rrow[:])
                    nc.vector.tensor_tensor(
                        ctxT[half, h // 2, :], psum_c[0:DH, :], rb[:], OP.mult
                    )

            attn_pools[1].__exit__(None, None, None)
            attn_pools[0].__exit__(None, None, None)

            # ---- stage 4: proj + residual (ch-outer over m-groups) ----
            st4_pool_cm = tc.tile_pool(name="ps_mm4", bufs=5, space="PSUM")
            ps_mlp = st4_pool_cm.__enter__()
            x2 = bigs.tile([P, CH, TOK], F32, tag="x2")

            def proj_group(ms):
                ws, psums = [], []
                for m in ms:
                    w = wpool.tile([P, CH, P], F32R, tag="wcol8")
                    nc.sync.dma_start(
                        w[:],
                        proj_wT[:, m * P : (m + 1) * P].rearrange(
                            "(ch p) o -> p ch o", p=P
                        ),
                    )
                    ws.append(w)
                    psums.append(ps_mlp.tile([P, TOK], F32, tag="acc", name=f"ps_{m}"))
                for ch in range(CH):
                    for i in range(len(ms)):
                        nc.tensor.matmul(
                            psums[i][:],
                            ws[i][:, ch, :],
                            ctxT[:, ch, :],
                            start=(ch == 0),
                            stop=(ch == CH - 1),
                        )
                for i, m in enumerate(ms):
                    attn_sb = stg.tile([P, TOK], F32, tag="stg", bufs=2)
                    nc.scalar.activation(
                        out=attn_sb[:],
                        in_=psums[i][:],
                        func=AF.Identity,
                        bias=projb_sb[:, m : m + 1],
                    )
                    xc = work.tile([P, TOK], F32, tag="xc")
                    nc.sync.dma_start(xc[:], xT_chunks[:, m, :])
                    nc.vector.tensor_add(out=x2[:, m, :], in0=attn_sb[:], in1=xc[:])

            if "proj" in stages:
                for g0 in range(CH):
                    proj_group(range(g0, g0 + 1))

            # ---- stage 5: LN2 stats only (affine folded into fc1 weights) ----
            x2r = bigs.tile([P, CH, TOK], F32R, tag="h12")
            rstd2_b = c2_b = None
            if "ln2" in stages:
                with tc.tile_pool(name="ps_row2", bufs=2, space="PSUM") as prow:
                    rstd2_b, c2_b = layer_norm(
                        lambda ch: x2[:, ch, :], ln2g_sb, ln2b_sb, None, prow,
                        xr_dst=x2r,
                    )

            # ---- stage 6: MLP in hidden-quarters with SBUF accumulator ----
            acc_sb = bigs.tile([P, CH, TOK], F32, tag="ctxacc")
            QH = 8  # hidden chunks per quarter
            for q in range(HCH // QH) if "mlp" in stages else ():
                g8 = bigs.tile([P, QH, TOK], F32R, tag="qg8")
                for mg in range(QH):
                    ws, psums = [], []
                    for i in range(1):
                        m = q * QH + mg + i
                        w = wpool.tile([P, CH, P], F32R, tag="wcol8")
                        nc.gpsimd.dma_start(
                            w[:],
                            fc1_wT[:, m * P : (m + 1) * P].rearrange(
                                "(ch p) o -> p ch o", p=P
                            ),
                        )
                        ws.append(w)
                        psums.append(ps_mlp.tile([P, TOK], F32, tag="acc", name=f"ps_{m}"))
                    for ch in range(CH):
                        for i in range(1):
                            nc.tensor.matmul(
                                psums[i][:],
                                ws[i][:, ch, :],
                                x2r[:, ch, :],
                                start=(ch == 0),
                                stop=(ch == CH - 1),
                            )
                    for i in range(1):
                        m = q * QH + mg + i
                        tmp = work.tile([P, TOK], F32, tag="t1", name=f"cor_{m}")
                        nc.vector.tensor_scalar(
                            out=tmp[:],
                            in0=c2_b[:],
                            scalar1=fc1mg_sb[:, m : m + 1],
                            scalar2=None,
                            op0=OP.mult,
                        )
                        t1 = work.tile([P, TOK], F32, tag="t1", name=f"t1_{m}")
                        nc.vector.tensor_tensor(
                            t1[:], psums[i][:], rstd2_b[:], OP.mult
                        )
                        nc.vector.tensor_add(t1[:], t1[:], tmp[:])
                        nc.scalar.activation(
                            out=g8[:, mg + i, :],
                            in_=t1[:],
                            func=AF.Gelu,
                            bias=fc1b_sb[:, m : m + 1],
                        )
                for m2g in range(CH):
                    ws2, psums2 = [], []
                    for i in range(1):
                        m2 = m2g + i
                        w2 = wpool.tile([P, QH, P], F32R, tag="w8", bufs=3)
                        nc.gpsimd.dma_start(
                            w2[:],
                            fc2_wT[
                                q * QH * P : (q + 1) * QH * P, m2 * P : (m2 + 1) * P
                            ].rearrange("(hc p) o -> p hc o", p=P),
                        )
                        ws2.append(w2)
                        psums2.append(ps_mlp.tile([P, TOK], F32, tag="acc", name=f"ps2_{m2}"))
                    for hc in range(QH):
                        for i in range(1):
                            nc.tensor.matmul(
                                psums2[i][:],
                                ws2[i][:, hc, :],
                                g8[:, hc, :],
                                start=(hc == 0),
                                stop=(hc == QH - 1),
                            )
                    for i in range(1):
                        m2 = m2g + i
                        if q == 0:
                            nc.vector.tensor_copy(
                                out=acc_sb[:, m2, :], in_=psums2[i][:]
                            )
                        elif q < HCH // QH - 1:
                            nc.vector.tensor_add(
                                out=acc_sb[:, m2, :],
                                in0=acc_sb[:, m2, :],
                                in1=psums2[i][:],
                            )
                        else:
                            o_sb = stg.tile([P, TOK], F32, tag="stg", bufs=2)
                            nc.scalar.activation(
                                out=o_sb[:],
                                in_=psums2[i][:],
                                func=AF.Identity,
                                bias=fc2b_sb[:, m2 : m2 + 1],
                            )
                            nc.vector.tensor_add(
                                out=o_sb[:], in0=o_sb[:], in1=acc_sb[:, m2, :]
                            )
                            o_f = stg.tile([P, TOK], F32, tag="of", bufs=2)
                            nc.vector.tensor_add(
                                out=o_f[:], in0=o_sb[:], in1=x2[:, m2, :]
                            )
                            nc.sync.dma_start(
                                outT[m2 * P : (m2 + 1) * P, :], o_f[:]
                            )
            if "mlp" not in stages:
                o_f = stg.tile([P, TOK], F32, tag="of", bufs=2)
                nc.vector.tensor_copy(out=o_f[:], in_=x2[:, 0, :])
                nc.sync.dma_start(outT[0:P, :], o_f[:])
            st4_pool_cm.__exit__(None, None, None)

    if do_compile:
        nc.compile()
    return nc


def build_program_ablated(stages):
    return build_program(stages=stages, do_compile=False)


_CACHE = {}


def _get_program():
    if "nc" not in _CACHE:
        _CACHE["nc"] = build_program()
    return _CACHE["nc"]


def _prep_inputs(inputs):
    """Host-side sharding + layout prep. Returns per-core in_maps."""
    x = np.asarray(inputs["x"], dtype=np.float32)
    shared = {
        "qkv_wT": round_fp32r(np.asarray(inputs["qkv_w"], np.float32).T),
        "proj_wT": round_fp32r(np.asarray(inputs["proj_w"], np.float32).T),
        "fc1_wT": round_fp32r(
            (
                np.asarray(inputs["fc1_w"], np.float32)
                * np.asarray(inputs["ln2_g"], np.float32)[None, :]
            ).T
        ),
        "fc2_wT": round_fp32r(np.asarray(inputs["fc2_w"], np.float32).T),
        "ln1g": _ln_stripe(inputs["ln1_g"]),
        "ln1b": _ln_stripe(inputs["ln1_b"]),
        "ln2g": _ln_stripe(inputs["ln2_g"]),
        "ln2b": _ln_stripe(inputs["ln2_b"]),
        "projb": _ln_stripe(inputs["proj_b"]),
        "fc1b": _ln_stripe(
            np.asarray(inputs["fc1_b"], np.float32)
            + np.asarray(inputs["fc1_w"], np.float32)
            @ np.asarray(inputs["ln2_b"], np.float32)
        ),
        "fc1mg": _ln_stripe(
            -(
                np.asarray(inputs["fc1_w"], np.float32)
                @ np.asarray(inputs["ln2_g"], np.float32)
            )
        ),
        "fc2b": _ln_stripe(inputs["fc2_b"]),
    }
    in_maps = []
    for c in range(NCORES):
        b, blk = divmod(c, RANKS)
        xblk = x[b, blk * TOK : (blk + 1) * TOK, :]  # [TOK, D]
        m = dict(shared)
        m["xT"] = np.ascontiguousarray(xblk.T)  # [D, TOK]
        in_maps.append(m)
    return in_maps


def _assemble(results):
    out = np.empty((B, N, D), dtype=np.float32)
    for c in range(NCORES):
        b, blk = divmod(c, RANKS)
        out[b, blk * TOK : (blk + 1) * TOK, :] = results[c]["outT"].T
    return out


def run_device(inputs, **kwargs):
    nc = _get_program()
    in_maps = _prep_inputs(inputs)
    res = run_bass_kernel_spmd(nc, in_maps, core_ids=list(range(NCORES)), **kwargs)
    return _assemble(res.results), res


def kernel(**inputs) -> np.ndarray:
    out, _ = run_device(inputs)
    return out
